# revision 33
# baseline (speedup 1.0000x reference)
"""Trainium2 Bass kernel for nn_Attention_81776177315877.

Separable-conv attention block (CMT/PVT style):
  x (B=8, 3136, 256) -> q/k/v = sepconv(dw3x3+BN+pw1x1, k/v stride 2)
  -> 8-head attention (d=32) -> proj.

Sharding: data-parallel over batch, core b <- batch b. No collectives.

Device strategy (per core, channel-major conv, key-major attention):
  - fold BN+depthwise taps into the pointwise weights on host: sepconv =
    sum over 9 taps of (W''_tap @ x_shifted) + const as PSUM-accumulated
    matmuls at full K=128 over a zero-padded channel-major image.
  - attention: S^T (keys on partitions) via 4-head tile_position row
    packing, softmax exp SPLIT between ScalarE (native exp, heads 0-1)
    and VectorE (1-op Schraudolph bit-trick exp -> int16 == bf16 bits,
    heads 2-3) so neither engine is the bottleneck; O^T and the softmax
    denominator via col-packed K=112 matmuls.
  - proj computed token-major (lhsT = channel-major O chunks) with the
    bias fused into the PSUM->SBUF copy, output DMA'd directly (no DRAM
    transpose round-trip).
"""

import sys

sys.path.insert(0, "/opt/trn_rl_repo")

import numpy as np
import ml_dtypes

import concourse.bass as bass
import concourse.bacc as bacc
import concourse.mybir as mybir
import concourse.tile as tile
from concourse.bass_utils import run_bass_kernel_spmd
from concourse.masks import make_identity

FP = mybir.dt.float32
BF = mybir.dt.bfloat16
I16 = mybir.dt.int16
F8 = mybir.dt.float8e4
AF = mybir.ActivationFunctionType
ALU = mybir.AluOpType
DR = mybir.MatmulPerfMode.DoubleRow
NP8 = mybir.dt.np(F8)          # ml_dtypes.float8_e4m3 (max finite 240)
PADW2 = 64                     # padded row length: 58*64 % 16 == 0 for DoubleRow

C = 256
HEADS = 8
D = 32
HH = 56
N = HH * HH          # 3136 query tokens
HK = 28
NK = HK * HK         # 784 key tokens
PADW = HH + 2        # 58
EPS = 1e-5
SCALE = D ** -0.5

IC_CH = 8            # query rows per chunk -> 448 free
IC_F = IC_CH * HH    # 448
N_IC = HH // IC_CH   # 7
KC_CH = 14           # k/v output rows per chunk -> 392 free
KC_F = KC_CH * HK    # 392
N_KC = HK // KC_CH   # 2
JT = 112             # key tile (partitions) for attention
N_JT = NK // JT      # 7

# Schraudolph exp-by-bits on DVE: int16(s*A + B) reinterpreted as bf16.
# e = exp(SCALE*s) ~ bits 128*(127 - DELTA + SCALE*s*log2(e))
LOG2E = 1.4426950408889634
SCH_DELTA = 0.0547                       # rms-optimal knot shift
SCH_A = 128.0 * LOG2E * SCALE
SCH_B = 128.0 * (127.0 - SCH_DELTA)
SC_HEADS = 2                             # heads on ScalarE per jt tile

N_TT = (N + 127) // 128  # 25 output token chunks

_CACHED = {}


def _build_nc():
    nc = bacc.Bacc("TRN2", target_bir_lowering=False, debug=False, num_devices=8)

    xp_d = nc.dram_tensor("x_pad8", [128, 2, PADW, PADW2], F8, kind="ExternalInput")
    w9t_d = {}
    const_d = {}
    ws_d = {}
    for p in ("q", "k", "v"):
        w9t_d[p] = nc.dram_tensor(f"{p}_w9t8", [128, 2, 9, C], F8, kind="ExternalInput")
        # packed per-projection fp32 row data: [const_cb0, const_cb1, ws_cb0, ws_cb1]
        const_d[p] = nc.dram_tensor(f"{p}_cw", [128, 4], FP, kind="ExternalInput")
    pw2_d = nc.dram_tensor("proj_w2", [2, 128, C], BF, kind="ExternalInput")
    bbc_d = nc.dram_tensor("bias_bc", [128, C], FP, kind="ExternalInput")
    out_d = nc.dram_tensor("out", [N, C], FP, kind="ExternalOutput")

    with tile.TileContext(nc) as tc:
        with (
            tc.tile_pool(name="persist", bufs=1) as pp,
            tc.tile_pool(name="ep", bufs=4) as ep,
            tc.tile_pool(name="rp", bufs=2) as rp,
            tc.tile_pool(name="op", bufs=3) as outp,
            tc.tile_pool(name="ps", bufs=2, space="PSUM") as psp,
            tc.tile_pool(name="pss", bufs=1, space="PSUM") as pss,
            tc.tile_pool(name="pso", bufs=1, space="PSUM") as pso,
            tc.tile_pool(name="psd", bufs=1, space="PSUM") as psd,
        ):
            ident = pp.tile([128, 128], FP, tag="ident", name="ident")
            make_identity(nc, ident[:])
            ones = pp.tile([128, 32], BF, tag="ones", name="ones")
            nc.gpsimd.memset(ones[:], 1.0)

            # ---- x arrives host-side padded/transposed as fp8 ----
            x_pad = pp.tile([128, 2, PADW, PADW2], F8, tag="xpad", name="xpad")
            nc.sync.dma_start(x_pad[:], xp_d[:, :, :, :])

            # ---- load folded weights (fp8, per-out-channel pow2 scaled) ----
            w9t = {}
            consts = {}
            ws = {}
            for p in ("k", "q", "v"):
                w9t[p] = pp.tile([128, 2, 9, C], F8, tag=f"w9t_{p}", name=f"w9t_{p}")
                nc.sync.dma_start(w9t[p][:], w9t_d[p][:, :, :, :])
                cw = pp.tile([128, 4], FP, tag=f"cw_{p}", name=f"cw_{p}")
                nc.sync.dma_start(cw[:], const_d[p][:, :])
                consts[p] = [cw[:, cb:cb + 1] for cb in range(2)]
                ws[p] = [cw[:, 2 + cb:3 + cb] for cb in range(2)]
            pw2 = [pp.tile([128, C], BF, tag=f"pw2{cb}", name=f"pw2{cb}") for cb in range(2)]
            for cb in range(2):
                nc.sync.dma_start(pw2[cb][:], pw2_d[cb, :, :])
            bias_bc = pp.tile([128, C], FP, tag="bbc", name="bbc")
            nc.sync.dma_start(bias_bc[:], bbc_d[:, :])

            # ---- conv helper: sepconv as 9 DoubleRow (K=256) matmuls ----
            def conv_chunk(p, dst_tiles, stride, ch_rows, wo, ch_idx, eng=None):
                # output rows [ch_idx*ch_rows, ...), all wo cols
                fsz = ch_rows * wo
                for cbo in range(2):
                    cps = psp.tile([128, 512], FP, tag="ps", name="ps")
                    for tap in range(9):
                        dh, dw = tap // 3 - 1, tap % 3 - 1
                        r0 = 1 + stride * ch_idx * ch_rows + dh
                        c0 = 1 + dw
                        if stride == 1:
                            rhs = x_pad[:, :, r0:r0 + ch_rows, c0:c0 + wo]
                        else:
                            xp2 = x_pad[:].rearrange(
                                "p k (ho a) (wv b) -> p k ho a wv b", a=2, b=2
                            )
                            rhs = xp2[
                                :,
                                :,
                                r0 // 2: r0 // 2 + ch_rows,
                                r0 % 2,
                                c0 // 2: c0 // 2 + wo,
                                c0 % 2,
                            ]
                        nc.tensor.matmul(
                            cps[:, :fsz],
                            lhsT=(w9t[p][:, :, tap, cbo * 128:(cbo + 1) * 128]),
                            rhs=(rhs),
                            start=(tap == 0),
                            stop=(tap == 8),
                            perf_mode=DR,
                        )
                    # fused: undo the fp8 weight scaling + add the folded const
                    if eng == "scalar":
                        nc.scalar.activation(
                            dst_tiles[cbo][:, ch_idx * fsz:(ch_idx + 1) * fsz],
                            cps[:, :fsz],
                            AF.Identity,
                            bias=consts[p][cbo],
                            scale=ws[p][cbo],
                        )
                    else:
                        nc.vector.tensor_scalar(
                            dst_tiles[cbo][:, ch_idx * fsz:(ch_idx + 1) * fsz],
                            cps[:, :fsz],
                            ws[p][cbo],
                            consts[p][cbo],
                            ALU.mult,
                            ALU.add,
                        )

            # ---- k conv, q ic=0 conv (so attention starts early), v conv ----
            k_cm = [pp.tile([128, NK], BF, tag=f"kcm{cb}", name=f"kcm{cb}") for cb in range(2)]
            v_cm = [pp.tile([128, NK], FP, tag=f"vcm{cb}", name=f"vcm{cb}") for cb in range(2)]
            q_cm = [pp.tile([128, N], BF, tag=f"qcm{cb}", name=f"qcm{cb}") for cb in range(2)]
            for ch in range(N_KC):
                conv_chunk("k", k_cm, 2, KC_CH, HK, ch)
            conv_chunk("q", q_cm, 1, IC_CH, HH, 0)
            for ch in range(N_KC):
                conv_chunk("v", v_cm, 2, KC_CH, HK, ch)
            v_tm = pp.tile([128, N_JT, C], BF, tag="vtm", name="vtm")
            for jt in range(N_JT):
                for cb in range(2):
                    tp = psp.tile([128, 512], FP, tag="ps", name="ps")
                    nc.tensor.transpose(
                        tp[:JT, :128],
                        v_cm[cb][:, jt * JT:(jt + 1) * JT],
                        ident[:],
                    )
                    nc.vector.tensor_copy(
                        v_tm[:JT, jt, cb * 128:(cb + 1) * 128], tp[:JT, :128]
                    )

            # ---- phase 2: attention, next q conv chunk prefetched inside ----
            o_cm = [pp.tile([128, N], BF, tag=f"ocm{cb}", name=f"ocm{cb}") for cb in range(2)]
            ti_done = 0
            for ic in range(N_IC):
                for hg in range(2):
                    o_ps = pso.tile([128, IC_F], FP, tag="o", name="o")
                    d_ps = psd.tile([128, IC_F], FP, tag="d", name="d")

                    def s_mm(jt):
                        # one PSUM tile (1 bank) per head: each exp engine's
                        # pipeline (exp -> next S -> exp) is chained per head,
                        # so per-head tiles keep the four chains independent
                        st = [
                            pss.tile([128, 512], FP, tag=f"s{hh}", name=f"s{hh}")
                            for hh in range(4)
                        ]
                        for hh in (0, 2, 1, 3):
                            nc.tensor.matmul(
                                st[hh][:JT, :IC_F],
                                lhsT=(k_cm[hg][hh * 32:(hh + 1) * 32, jt * JT:(jt + 1) * JT]),
                                rhs=(q_cm[hg][hh * 32:(hh + 1) * 32, ic * IC_F:(ic + 1) * IC_F]),
                                start=True,
                                stop=True,
                                tile_position=(32 * hh, 0),
                            )
                        return st

                    st = s_mm(0)
                    for jt in range(N_JT):
                        # exp split: ScalarE native on heads 0..SC_HEADS-1,
                        # DVE Schraudolph bit-exp on the rest
                        et = [None] * 4
                        for hh in (0, 2, 1, 3):
                            e = ep.tile([128, IC_F], BF, tag=f"e{hh}", name=f"e{hh}")
                            if hh < SC_HEADS:
                                nc.scalar.activation(
                                    e[:JT, :], st[hh][:JT, :IC_F], AF.Exp, scale=SCALE
                                )
                            else:
                                nc.vector.tensor_scalar(
                                    e[:JT, :].bitcast(I16),
                                    st[hh][:JT, :IC_F],
                                    SCH_A,
                                    SCH_B,
                                    ALU.mult,
                                    ALU.add,
                                )
                            et[hh] = e
                        if jt + 1 < N_JT:
                            st = s_mm(jt + 1)
                        for hh in (0, 2, 1, 3):
                            nc.tensor.matmul(
                                o_ps[hh * 32:(hh + 1) * 32, :],
                                lhsT=(v_tm[:JT, jt, hg * 128 + hh * 32: hg * 128 + (hh + 1) * 32]),
                                rhs=(et[hh][:JT, :]),
                                start=(jt == 0),
                                stop=(jt == N_JT - 1),
                                tile_position=(0, 32 * hh),
                                skip_group_check=True,
                            )
                            nc.tensor.matmul(
                                d_ps[hh * 32:(hh + 1) * 32, :],
                                lhsT=(ones[:JT, :]),
                                rhs=(et[hh][:JT, :]),
                                start=(jt == 0),
                                stop=(jt == N_JT - 1),
                                tile_position=(0, 32 * hh),
                                skip_group_check=True,
                            )
                    # free o_ps ASAP: ScalarE copies PSUM->SBUF (it idles at
                    # the boundary), DVE then normalizes from SBUF
                    r_t = rp.tile([128, IC_F], FP, tag="r", name="r")
                    nc.vector.reciprocal_approx_fast(r_t[:], d_ps[:])
                    ocp = rp.tile([128, IC_F], BF, tag="ocp", name="ocp")
                    nc.scalar.copy(ocp[:], o_ps[:])
                    nc.vector.tensor_mul(
                        o_cm[hg][:, ic * IC_F:(ic + 1) * IC_F], ocp[:], r_t[:]
                    )
                # prefetch next q conv chunk; bias-add on ScalarE so the
                # DVE FIFO (Schraudolph exps) is not blocked behind it
                if ic + 1 < N_IC:
                    conv_chunk("q", q_cm, 1, IC_CH, HH, ic + 1, eng="scalar")

                # ---- token-major proj for tokens that are ready ----
                while ti_done < N_TT and min(ti_done * 128, N - 128) + 128 <= (ic + 1) * IC_F:
                    st = min(ti_done * 128, N - 128)  # overlap the ragged tail
                    po = psp.tile([128, 512], FP, tag="ps", name="ps")
                    for cb in range(2):
                        nc.tensor.matmul(
                            po[:, :C],
                            lhsT=(o_cm[cb][:, st: st + 128]),
                            rhs=(pw2[cb][:]),
                            start=(cb == 0),
                            stop=(cb == 1),
                        )
                    ot = outp.tile([128, C], FP, tag="ot", name="ot")
                    # fused bias add with the PSUM->SBUF copy
                    nc.vector.scalar_tensor_tensor(
                        ot[:], po[:, :C], 1.0, bias_bc[:], ALU.mult, ALU.add
                    )
                    nc.sync.dma_start(out_d[st: st + 128, :], ot[:])
                    ti_done += 1

    nc.compile()
    return nc


def _fold_weights(inp, p):
    scale = inp[f"{p}_bn_g"] / np.sqrt(inp[f"{p}_bn_v"] + EPS)
    shift = inp[f"{p}_bn_b"] - inp[f"{p}_bn_m"] * scale
    w2 = inp[f"{p}_pw_w"] * scale[None, :]          # (o, c)
    w9 = inp[f"{p}_dw_w"].reshape(C, 9)             # (c, tap)
    w9t = np.ascontiguousarray(
        w2.T[None, :, :] * w9.T[:, :, None]          # (tap, c, o)
    ).astype(np.float32)
    const = (
        inp[f"{p}_pw_w"] @ (scale * inp[f"{p}_dw_b"] + shift) + inp[f"{p}_pw_b"]
    ).astype(np.float32)
    return w9t, const.reshape(C, 1)


def _host_inputs(inp):
    common = {}
    for p in ("q", "k", "v"):
        w9t, const = _fold_weights(inp, p)        # (tap, c, o), (C, 1)
        # layout for DoubleRow: [c_in_block(128), cbi(2), tap(9), o(256)]
        w9l = np.ascontiguousarray(
            w9t.reshape(9, 2, 128, C).transpose(2, 1, 0, 3)
        )
        # per-output-channel pow2 scale so fp8e4 (max 240) keeps precision
        mx = np.abs(w9l).max(axis=(0, 1, 2))      # (256,)
        sc = np.exp2(np.floor(np.log2(200.0 / np.maximum(mx, 1e-30))))
        common[f"{p}_w9t8"] = (w9l * sc[None, None, None, :]).astype(NP8)
        # packed [const_cb0, const_cb1, ws_cb0, ws_cb1] as [128, 4] fp32
        cw = np.empty((128, 4), np.float32)
        cw[:, 0:2] = const.reshape(2, 128).T
        cw[:, 2:4] = (1.0 / sc).reshape(2, 128).T
        common[f"{p}_cw"] = cw
    pwt = np.ascontiguousarray(inp["proj_w"].T)      # (c_in, c_out)
    common["proj_w2"] = np.ascontiguousarray(
        pwt.reshape(2, 128, C)
    ).astype(ml_dtypes.bfloat16)
    common["bias_bc"] = np.ascontiguousarray(
        np.broadcast_to(inp["proj_b"].astype(np.float32), (128, C))
    )
    return common


def _x_pad8(xb):
    # (3136, 256) -> fp8 [c_in_block(128), cbi(2), 58, 64] zero-padded image
    xc = xb.reshape(HH, HH, C).transpose(2, 0, 1)    # (256, 56, 56)
    xp = np.zeros((2, 128, PADW, PADW2), np.float32)
    xp[0, :, 1:57, 1:57] = xc[:128]
    xp[1, :, 1:57, 1:57] = xc[128:]
    return np.ascontiguousarray(xp.transpose(1, 0, 2, 3)).astype(NP8)


def _in_maps(inp):
    common = _host_inputs(inp)
    x = np.asarray(inp["x"]).astype(np.float32)
    return [dict(common, x_pad8=_x_pad8(x[b])) for b in range(x.shape[0])]


def kernel(**inputs):
    inp = {k: np.asarray(v) for k, v in inputs.items()}
    B = inp["x"].shape[0]

    if "nc" not in _CACHED:
        _CACHED["nc"] = _build_nc()
    nc = _CACHED["nc"]

    in_maps = _in_maps(inp)
    res = run_bass_kernel_spmd(nc, in_maps, list(range(B)))
    out = np.stack([res.results[b]["out"] for b in range(B)], axis=0)
    return out.astype(np.float32)


# revision 34
# speedup vs baseline: 1.1994x; 1.1994x over previous
"""Trainium2 Bass kernel for nn_Attention_81776177315877.

Separable-conv attention block (CMT/PVT style):
  x (B=8, 3136, 256) -> q/k/v = sepconv(dw3x3+BN+pw1x1, k/v stride 2)
  -> 8-head attention (d=32) -> proj.

Sharding: data-parallel over batch, core b <- batch b. No collectives.

Device strategy (per core, channel-major conv, key-major attention):
  - fold BN+depthwise taps into the pointwise weights on host: sepconv =
    sum over 9 taps of (W''_tap @ x_shifted) + const as PSUM-accumulated
    matmuls at full K=128 over a zero-padded channel-major image.
  - attention: S^T (keys on partitions) via 4-head tile_position row
    packing, softmax exp SPLIT between ScalarE (native exp, heads 0-1)
    and VectorE (1-op Schraudolph bit-trick exp -> int16 == bf16 bits,
    heads 2-3) so neither engine is the bottleneck; O^T and the softmax
    denominator via col-packed K=112 matmuls.
  - proj computed token-major (lhsT = channel-major O chunks) with the
    bias fused into the PSUM->SBUF copy, output DMA'd directly (no DRAM
    transpose round-trip).
"""

import sys

sys.path.insert(0, "/opt/trn_rl_repo")

import numpy as np
import ml_dtypes

import concourse.bass as bass
import concourse.bacc as bacc
import concourse.mybir as mybir
import concourse.tile as tile
from concourse.bass_utils import run_bass_kernel_spmd
from concourse.masks import make_identity

FP = mybir.dt.float32
BF = mybir.dt.bfloat16
I16 = mybir.dt.int16
F8 = mybir.dt.float8e4
AF = mybir.ActivationFunctionType
ALU = mybir.AluOpType
DR = mybir.MatmulPerfMode.DoubleRow
NP8 = mybir.dt.np(F8)          # ml_dtypes.float8_e4m3 (max finite 240)
PADW2 = 64                     # padded row length: 58*64 % 16 == 0 for DoubleRow

C = 256
HEADS = 8
D = 32
HH = 56
N = HH * HH          # 3136 query tokens
HK = 28
NK = HK * HK         # 784 key tokens
PADW = HH + 2        # 58
EPS = 1e-5
SCALE = D ** -0.5

IC_CH = 8            # query rows per chunk -> 448 free
IC_F = IC_CH * HH    # 448
N_IC = HH // IC_CH   # 7
KC_CH = 14           # k/v output rows per chunk -> 392 free
KC_F = KC_CH * HK    # 392
N_KC = HK // KC_CH   # 2
JT = 112             # key tile (partitions) for attention
N_JT = NK // JT      # 7

# Schraudolph exp-by-bits on DVE: int16(s*A + B) reinterpreted as bf16.
# e = exp(SCALE*s) ~ bits 128*(127 - DELTA + SCALE*s*log2(e))
LOG2E = 1.4426950408889634
SCH_DELTA = 0.0547                       # rms-optimal knot shift
SCH_A = 128.0 * LOG2E * SCALE
SCH_B = 128.0 * (127.0 - SCH_DELTA)
SC_HEADS = 2                             # heads on ScalarE per jt tile

N_TT = (N + 127) // 128  # 25 output token chunks

_CACHED = {}


def _build_nc():
    nc = bacc.Bacc("TRN2", target_bir_lowering=False, debug=False, num_devices=8)

    xp_d = nc.dram_tensor("x_pad8", [128, 2, PADW, PADW2], F8, kind="ExternalInput")
    w9t_d = {}
    const_d = {}
    ws_d = {}
    for p in ("q", "k", "v"):
        w9t_d[p] = nc.dram_tensor(f"{p}_w9t8", [128, 2, 9, C], F8, kind="ExternalInput")
        # packed per-projection fp32 row data: [const_cb0, const_cb1, ws_cb0, ws_cb1]
        const_d[p] = nc.dram_tensor(f"{p}_cw", [128, 4], FP, kind="ExternalInput")
    pw2_d = nc.dram_tensor("proj_w2", [2, 128, C], BF, kind="ExternalInput")
    bbc_d = nc.dram_tensor("bias_bc", [128, C], FP, kind="ExternalInput")
    out_d = nc.dram_tensor("out", [N, C], FP, kind="ExternalOutput")

    with tile.TileContext(nc) as tc:
        with (
            tc.tile_pool(name="persist", bufs=1) as pp,
            tc.tile_pool(name="ep", bufs=4) as ep,
            tc.tile_pool(name="rp", bufs=2) as rp,
            tc.tile_pool(name="op", bufs=3) as outp,
            tc.tile_pool(name="ps", bufs=2, space="PSUM") as psp,
            tc.tile_pool(name="pss", bufs=1, space="PSUM") as pss,
            tc.tile_pool(name="pso", bufs=1, space="PSUM") as pso,
            tc.tile_pool(name="psd", bufs=1, space="PSUM") as psd,
        ):
            ident = pp.tile([128, 128], FP, tag="ident", name="ident")
            make_identity(nc, ident[:])
            ones = pp.tile([128, 32], BF, tag="ones", name="ones")
            nc.gpsimd.memset(ones[:], 1.0)

            # ---- x arrives host-side padded/transposed as fp8 ----
            x_pad = pp.tile([128, 2, PADW, PADW2], F8, tag="xpad", name="xpad")
            nc.sync.dma_start(x_pad[:], xp_d[:, :, :, :])

            # ---- load folded weights (fp8, per-out-channel pow2 scaled) ----
            w9t = {}
            consts = {}
            ws = {}
            for p in ("k", "q", "v"):
                w9t[p] = pp.tile([128, 2, 9, C], F8, tag=f"w9t_{p}", name=f"w9t_{p}")
                nc.sync.dma_start(w9t[p][:], w9t_d[p][:, :, :, :])
                cw = pp.tile([128, 4], FP, tag=f"cw_{p}", name=f"cw_{p}")
                nc.sync.dma_start(cw[:], const_d[p][:, :])
                consts[p] = [cw[:, cb:cb + 1] for cb in range(2)]
                ws[p] = [cw[:, 2 + cb:3 + cb] for cb in range(2)]
            pw2 = [pp.tile([128, C], BF, tag=f"pw2{cb}", name=f"pw2{cb}") for cb in range(2)]
            for cb in range(2):
                nc.sync.dma_start(pw2[cb][:], pw2_d[cb, :, :])
            bias_bc = pp.tile([128, C], FP, tag="bbc", name="bbc")
            nc.sync.dma_start(bias_bc[:], bbc_d[:, :])

            # ---- conv helper: sepconv as 9 DoubleRow (K=256) matmuls ----
            def conv_chunk(p, dst_tiles, stride, ch_rows, wo, ch_idx, eng=None):
                # output rows [ch_idx*ch_rows, ...), all wo cols
                fsz = ch_rows * wo
                for cbo in range(2):
                    cps = psp.tile([128, 512], FP, tag="ps", name="ps")
                    for tap in range(9):
                        dh, dw = tap // 3 - 1, tap % 3 - 1
                        r0 = 1 + stride * ch_idx * ch_rows + dh
                        c0 = 1 + dw
                        if stride == 1:
                            rhs = x_pad[:, :, r0:r0 + ch_rows, c0:c0 + wo]
                        else:
                            xp2 = x_pad[:].rearrange(
                                "p k (ho a) (wv b) -> p k ho a wv b", a=2, b=2
                            )
                            rhs = xp2[
                                :,
                                :,
                                r0 // 2: r0 // 2 + ch_rows,
                                r0 % 2,
                                c0 // 2: c0 // 2 + wo,
                                c0 % 2,
                            ]
                        nc.tensor.matmul(
                            cps[:, :fsz],
                            lhsT=(w9t[p][:, :, tap, cbo * 128:(cbo + 1) * 128]),
                            rhs=(rhs),
                            start=(tap == 0),
                            stop=(tap == 8),
                            perf_mode=DR,
                        )
                    # fused: undo the fp8 weight scaling + add the folded const
                    if eng == "scalar":
                        nc.scalar.activation(
                            dst_tiles[cbo][:, ch_idx * fsz:(ch_idx + 1) * fsz],
                            cps[:, :fsz],
                            AF.Identity,
                            bias=consts[p][cbo],
                            scale=ws[p][cbo],
                        )
                    else:
                        nc.vector.tensor_scalar(
                            dst_tiles[cbo][:, ch_idx * fsz:(ch_idx + 1) * fsz],
                            cps[:, :fsz],
                            ws[p][cbo],
                            consts[p][cbo],
                            ALU.mult,
                            ALU.add,
                        )

            # ---- k conv, q ic=0 conv (so attention starts early), v conv ----
            k_cm = [pp.tile([128, NK], BF, tag=f"kcm{cb}", name=f"kcm{cb}") for cb in range(2)]
            v_cm = [pp.tile([128, NK], FP, tag=f"vcm{cb}", name=f"vcm{cb}") for cb in range(2)]
            q_cm = [pp.tile([128, N], BF, tag=f"qcm{cb}", name=f"qcm{cb}") for cb in range(2)]
            for ch in range(N_KC):
                conv_chunk("k", k_cm, 2, KC_CH, HK, ch)
            conv_chunk("q", q_cm, 1, IC_CH, HH, 0)
            for ch in range(N_KC):
                conv_chunk("v", v_cm, 2, KC_CH, HK, ch)
            v_tm = pp.tile([128, N_JT, C], BF, tag="vtm", name="vtm")
            for jt in range(N_JT):
                for cb in range(2):
                    tp = psp.tile([128, 512], FP, tag="ps", name="ps")
                    nc.tensor.transpose(
                        tp[:JT, :128],
                        v_cm[cb][:, jt * JT:(jt + 1) * JT],
                        ident[:],
                    )
                    nc.vector.tensor_copy(
                        v_tm[:JT, jt, cb * 128:(cb + 1) * 128], tp[:JT, :128]
                    )

            # ---- phase 2: attention, next q conv chunk prefetched inside ----
            o_cm = [pp.tile([128, N], BF, tag=f"ocm{cb}", name=f"ocm{cb}") for cb in range(2)]
            ti_done = 0
            for ic in range(N_IC):
                for hg in range(2):
                    o_ps = pso.tile([128, IC_F], FP, tag="o", name="o")
                    d_ps = psd.tile([128, IC_F], FP, tag="d", name="d")

                    def s_mm(jt):
                        # one PSUM tile (1 bank) per head: each exp engine's
                        # pipeline (exp -> next S -> exp) is chained per head,
                        # so per-head tiles keep the four chains independent
                        st = [
                            pss.tile([128, 512], FP, tag=f"s{hh}", name=f"s{hh}")
                            for hh in range(4)
                        ]
                        for hh in (0, 2, 1, 3):
                            nc.tensor.matmul(
                                st[hh][:JT, :IC_F],
                                lhsT=(k_cm[hg][hh * 32:(hh + 1) * 32, jt * JT:(jt + 1) * JT]),
                                rhs=(q_cm[hg][hh * 32:(hh + 1) * 32, ic * IC_F:(ic + 1) * IC_F]),
                                start=True,
                                stop=True,
                                tile_position=(32 * hh, 0),
                            )
                        return st

                    st = s_mm(0)
                    for jt in range(N_JT):
                        # exp split: ScalarE native on heads 0..SC_HEADS-1,
                        # DVE Schraudolph bit-exp on the rest
                        et = [None] * 4
                        for hh in (0, 2, 1, 3):
                            e = ep.tile([128, IC_F], BF, tag=f"e{hh}", name=f"e{hh}")
                            if hh < SC_HEADS:
                                nc.scalar.activation(
                                    e[:JT, :], st[hh][:JT, :IC_F], AF.Exp, scale=SCALE
                                )
                            else:
                                nc.vector.tensor_scalar(
                                    e[:JT, :].bitcast(I16),
                                    st[hh][:JT, :IC_F],
                                    SCH_A,
                                    SCH_B,
                                    ALU.mult,
                                    ALU.add,
                                )
                            et[hh] = e
                        if jt + 1 < N_JT:
                            st = s_mm(jt + 1)
                        for hh in (0, 2, 1, 3):
                            nc.tensor.matmul(
                                o_ps[hh * 32:(hh + 1) * 32, :],
                                lhsT=(v_tm[:JT, jt, hg * 128 + hh * 32: hg * 128 + (hh + 1) * 32]),
                                rhs=(et[hh][:JT, :]),
                                start=(jt == 0),
                                stop=(jt == N_JT - 1),
                                tile_position=(0, 32 * hh),
                                skip_group_check=True,
                            )
                            nc.tensor.matmul(
                                d_ps[hh * 32:(hh + 1) * 32, :],
                                lhsT=(ones[:JT, :]),
                                rhs=(et[hh][:JT, :]),
                                start=(jt == 0),
                                stop=(jt == N_JT - 1),
                                tile_position=(0, 32 * hh),
                                skip_group_check=True,
                            )
                    r_t = rp.tile([128, IC_F], FP, tag="r", name="r")
                    nc.vector.reciprocal_approx_fast(r_t[:], d_ps[:])
                    nc.vector.tensor_mul(
                        o_cm[hg][:, ic * IC_F:(ic + 1) * IC_F], o_ps[:], r_t[:]
                    )
                # prefetch next q conv chunk; bias-add on ScalarE so the
                # DVE FIFO (Schraudolph exps) is not blocked behind it
                if ic + 1 < N_IC:
                    conv_chunk("q", q_cm, 1, IC_CH, HH, ic + 1, eng="scalar")

                # ---- token-major proj for tokens that are ready ----
                while ti_done < N_TT and min(ti_done * 128, N - 128) + 128 <= (ic + 1) * IC_F:
                    st = min(ti_done * 128, N - 128)  # overlap the ragged tail
                    po = psp.tile([128, 512], FP, tag="ps", name="ps")
                    for cb in range(2):
                        nc.tensor.matmul(
                            po[:, :C],
                            lhsT=(o_cm[cb][:, st: st + 128]),
                            rhs=(pw2[cb][:]),
                            start=(cb == 0),
                            stop=(cb == 1),
                        )
                    ot = outp.tile([128, C], FP, tag="ot", name="ot")
                    # fused bias add with the PSUM->SBUF copy
                    nc.vector.scalar_tensor_tensor(
                        ot[:], po[:, :C], 1.0, bias_bc[:], ALU.mult, ALU.add
                    )
                    nc.sync.dma_start(out_d[st: st + 128, :], ot[:])
                    ti_done += 1

    nc.compile()
    return nc


def _fold_weights(inp, p):
    scale = inp[f"{p}_bn_g"] / np.sqrt(inp[f"{p}_bn_v"] + EPS)
    shift = inp[f"{p}_bn_b"] - inp[f"{p}_bn_m"] * scale
    w2 = inp[f"{p}_pw_w"] * scale[None, :]          # (o, c)
    w9 = inp[f"{p}_dw_w"].reshape(C, 9)             # (c, tap)
    w9t = np.ascontiguousarray(
        w2.T[None, :, :] * w9.T[:, :, None]          # (tap, c, o)
    ).astype(np.float32)
    const = (
        inp[f"{p}_pw_w"] @ (scale * inp[f"{p}_dw_b"] + shift) + inp[f"{p}_pw_b"]
    ).astype(np.float32)
    return w9t, const.reshape(C, 1)


def _host_inputs(inp):
    common = {}
    for p in ("q", "k", "v"):
        w9t, const = _fold_weights(inp, p)        # (tap, c, o), (C, 1)
        # layout for DoubleRow: [c_in_block(128), cbi(2), tap(9), o(256)]
        w9l = np.ascontiguousarray(
            w9t.reshape(9, 2, 128, C).transpose(2, 1, 0, 3)
        )
        # per-output-channel pow2 scale so fp8e4 (max 240) keeps precision
        mx = np.abs(w9l).max(axis=(0, 1, 2))      # (256,)
        sc = np.exp2(np.floor(np.log2(200.0 / np.maximum(mx, 1e-30))))
        common[f"{p}_w9t8"] = (w9l * sc[None, None, None, :]).astype(NP8)
        # packed [const_cb0, const_cb1, ws_cb0, ws_cb1] as [128, 4] fp32
        cw = np.empty((128, 4), np.float32)
        cw[:, 0:2] = const.reshape(2, 128).T
        cw[:, 2:4] = (1.0 / sc).reshape(2, 128).T
        common[f"{p}_cw"] = cw
    pwt = np.ascontiguousarray(inp["proj_w"].T)      # (c_in, c_out)
    common["proj_w2"] = np.ascontiguousarray(
        pwt.reshape(2, 128, C)
    ).astype(ml_dtypes.bfloat16)
    common["bias_bc"] = np.ascontiguousarray(
        np.broadcast_to(inp["proj_b"].astype(np.float32), (128, C))
    )
    return common


def _x_pad8(xb):
    # (3136, 256) -> fp8 [c_in_block(128), cbi(2), 58, 64] zero-padded image
    xc = xb.reshape(HH, HH, C).transpose(2, 0, 1)    # (256, 56, 56)
    xp = np.zeros((2, 128, PADW, PADW2), np.float32)
    xp[0, :, 1:57, 1:57] = xc[:128]
    xp[1, :, 1:57, 1:57] = xc[128:]
    return np.ascontiguousarray(xp.transpose(1, 0, 2, 3)).astype(NP8)


def _in_maps(inp):
    common = _host_inputs(inp)
    x = np.asarray(inp["x"]).astype(np.float32)
    return [dict(common, x_pad8=_x_pad8(x[b])) for b in range(x.shape[0])]


def kernel(**inputs):
    inp = {k: np.asarray(v) for k, v in inputs.items()}
    B = inp["x"].shape[0]

    if "nc" not in _CACHED:
        _CACHED["nc"] = _build_nc()
    nc = _CACHED["nc"]

    in_maps = _in_maps(inp)
    res = run_bass_kernel_spmd(nc, in_maps, list(range(B)))
    out = np.stack([res.results[b]["out"] for b in range(B)], axis=0)
    return out.astype(np.float32)


# revision 37
# speedup vs baseline: 1.2175x; 1.0150x over previous
"""Trainium2 Bass kernel for nn_Attention_81776177315877.

Separable-conv attention block (CMT/PVT style):
  x (B=8, 3136, 256) -> q/k/v = sepconv(dw3x3+BN+pw1x1, k/v stride 2)
  -> 8-head attention (d=32) -> proj.

Sharding: data-parallel over batch, core b <- batch b. No collectives.

Device strategy (per core, channel-major conv, key-major attention):
  - fold BN+depthwise taps into the pointwise weights on host: sepconv =
    sum over 9 taps of (W''_tap @ x_shifted) + const as PSUM-accumulated
    matmuls at full K=128 over a zero-padded channel-major image.
  - attention: S^T (keys on partitions) via 4-head tile_position row
    packing, softmax exp SPLIT between ScalarE (native exp, heads 0-1)
    and VectorE (1-op Schraudolph bit-trick exp -> int16 == bf16 bits,
    heads 2-3) so neither engine is the bottleneck; O^T and the softmax
    denominator via col-packed K=112 matmuls.
  - proj computed token-major (lhsT = channel-major O chunks) with the
    bias fused into the PSUM->SBUF copy, output DMA'd directly (no DRAM
    transpose round-trip).
"""

import sys

sys.path.insert(0, "/opt/trn_rl_repo")

import numpy as np
import ml_dtypes

import concourse.bass as bass
import concourse.bacc as bacc
import concourse.mybir as mybir
import concourse.tile as tile
from concourse.bass_utils import run_bass_kernel_spmd
from concourse.masks import make_identity

FP = mybir.dt.float32
BF = mybir.dt.bfloat16
I16 = mybir.dt.int16
F8 = mybir.dt.float8e4
AF = mybir.ActivationFunctionType
ALU = mybir.AluOpType
DR = mybir.MatmulPerfMode.DoubleRow
NP8 = mybir.dt.np(F8)          # ml_dtypes.float8_e4m3 (max finite 240)
PADW2 = 64                     # padded row length: 58*64 % 16 == 0 for DoubleRow

C = 256
HEADS = 8
D = 32
HH = 56
N = HH * HH          # 3136 query tokens
HK = 28
NK = HK * HK         # 784 key tokens
PADW = HH + 2        # 58
EPS = 1e-5
SCALE = D ** -0.5

IC_CH = 8            # query rows per chunk -> 448 free
IC_F = IC_CH * HH    # 448
N_IC = HH // IC_CH   # 7
KC_CH = 14           # k/v output rows per chunk -> 392 free
KC_F = KC_CH * HK    # 392
N_KC = HK // KC_CH   # 2
JT = 112             # key tile (partitions) for attention
N_JT = NK // JT      # 7

# Schraudolph exp-by-bits on DVE: int16(s*A + B) reinterpreted as bf16.
# e = exp(SCALE*s) ~ bits 128*(127 - DELTA + SCALE*s*log2(e))
LOG2E = 1.4426950408889634
SCH_DELTA = 0.0547                       # rms-optimal knot shift
SCH_A = 128.0 * LOG2E * SCALE
SCH_B = 128.0 * (127.0 - SCH_DELTA)
SC_HEADS = 2                             # heads on ScalarE per jt tile

N_TT = (N + 127) // 128  # 25 output token chunks

_CACHED = {}


def _build_nc():
    nc = bacc.Bacc("TRN2", target_bir_lowering=False, debug=False, num_devices=8)

    xp_d = nc.dram_tensor("x_pad8", [128, 2, PADW, PADW2], F8, kind="ExternalInput")
    w9t_d = {}
    const_d = {}
    ws_d = {}
    for p in ("q", "k", "v"):
        w9t_d[p] = nc.dram_tensor(f"{p}_w9t8", [128, 2, 9, C], F8, kind="ExternalInput")
        # packed per-projection fp32 row data: [const_cb0, const_cb1, ws_cb0, ws_cb1]
        const_d[p] = nc.dram_tensor(f"{p}_cw", [128, 4], FP, kind="ExternalInput")
    pw2_d = nc.dram_tensor("proj_w2", [2, 128, C], BF, kind="ExternalInput")
    bbc_d = nc.dram_tensor("bias_bc", [128, C], FP, kind="ExternalInput")
    out_d = nc.dram_tensor("out", [N, C], FP, kind="ExternalOutput")

    with tile.TileContext(nc) as tc:
        with (
            tc.tile_pool(name="persist", bufs=1) as pp,
            tc.tile_pool(name="ep", bufs=4) as ep,
            tc.tile_pool(name="rp", bufs=2) as rp,
            tc.tile_pool(name="op", bufs=3) as outp,
            tc.tile_pool(name="ps", bufs=2, space="PSUM") as psp,
            tc.tile_pool(name="pss", bufs=1, space="PSUM") as pss,
            tc.tile_pool(name="pso", bufs=1, space="PSUM") as pso,
            tc.tile_pool(name="psd", bufs=1, space="PSUM") as psd,
        ):
            ident = pp.tile([128, 128], FP, tag="ident", name="ident")
            make_identity(nc, ident[:])
            ones = pp.tile([128, 32], BF, tag="ones", name="ones")
            nc.gpsimd.memset(ones[:], 1.0)

            # ---- x arrives host-side padded/transposed as fp8 ----
            x_pad = pp.tile([128, 2, PADW, PADW2], F8, tag="xpad", name="xpad")
            nc.sync.dma_start(x_pad[:], xp_d[:, :, :, :])

            # ---- load folded weights (fp8, per-out-channel pow2 scaled) ----
            w9t = {}
            consts = {}
            ws = {}
            for p in ("k", "q", "v"):
                w9t[p] = pp.tile([128, 2, 9, C], F8, tag=f"w9t_{p}", name=f"w9t_{p}")
                nc.sync.dma_start(w9t[p][:], w9t_d[p][:, :, :, :])
                cw = pp.tile([128, 4], FP, tag=f"cw_{p}", name=f"cw_{p}")
                nc.sync.dma_start(cw[:], const_d[p][:, :])
                consts[p] = [cw[:, cb:cb + 1] for cb in range(2)]
                ws[p] = [cw[:, 2 + cb:3 + cb] for cb in range(2)]
            # proj weights are only needed much later; one packed DMA
            pw2 = [pp.tile([128, C], BF, tag=f"pw2{cb}", name=f"pw2{cb}") for cb in range(2)]
            bias_bc = pp.tile([128, C], FP, tag="bbc", name="bbc")
            nc.sync.dma_start(bias_bc[:], bbc_d[:, :])
            for cb in range(2):
                nc.sync.dma_start(pw2[cb][:], pw2_d[cb, :, :])

            # ---- conv helper: sepconv as 9 DoubleRow (K=256) matmuls ----
            def conv_chunk(p, dst_tiles, stride, ch_rows, wo, ch_idx, eng=None, cbos=(0, 1)):
                # output rows [ch_idx*ch_rows, ...), all wo cols
                fsz = ch_rows * wo
                for cbo in cbos:
                    cps = psp.tile([128, 512], FP, tag="ps", name="ps")
                    for tap in range(9):
                        dh, dw = tap // 3 - 1, tap % 3 - 1
                        r0 = 1 + stride * ch_idx * ch_rows + dh
                        c0 = 1 + dw
                        if stride == 1:
                            rhs = x_pad[:, :, r0:r0 + ch_rows, c0:c0 + wo]
                        else:
                            xp2 = x_pad[:].rearrange(
                                "p k (ho a) (wv b) -> p k ho a wv b", a=2, b=2
                            )
                            rhs = xp2[
                                :,
                                :,
                                r0 // 2: r0 // 2 + ch_rows,
                                r0 % 2,
                                c0 // 2: c0 // 2 + wo,
                                c0 % 2,
                            ]
                        nc.tensor.matmul(
                            cps[:, :fsz],
                            lhsT=(w9t[p][:, :, tap, cbo * 128:(cbo + 1) * 128]),
                            rhs=(rhs),
                            start=(tap == 0),
                            stop=(tap == 8),
                            perf_mode=DR,
                        )
                    # fused: undo the fp8 weight scaling + add the folded const
                    if eng == "scalar":
                        nc.scalar.activation(
                            dst_tiles[cbo][:, ch_idx * fsz:(ch_idx + 1) * fsz],
                            cps[:, :fsz],
                            AF.Identity,
                            bias=consts[p][cbo],
                            scale=ws[p][cbo],
                        )
                    else:
                        nc.vector.tensor_scalar(
                            dst_tiles[cbo][:, ch_idx * fsz:(ch_idx + 1) * fsz],
                            cps[:, :fsz],
                            ws[p][cbo],
                            consts[p][cbo],
                            ALU.mult,
                            ALU.add,
                        )

            # ---- k conv, q ic=0 conv (so attention starts early), v conv ----
            k_cm = [pp.tile([128, NK], BF, tag=f"kcm{cb}", name=f"kcm{cb}") for cb in range(2)]
            v_cm = [pp.tile([128, NK], FP, tag=f"vcm{cb}", name=f"vcm{cb}") for cb in range(2)]
            q_cm = [pp.tile([128, N], BF, tag=f"qcm{cb}", name=f"qcm{cb}") for cb in range(2)]
            for ch in range(N_KC):
                conv_chunk("k", k_cm, 2, KC_CH, HK, ch)
            # q ic=0 cbo=0 first: hg=0 attention needs only that half, so its
            # S/exp pipeline starts while the v conv still streams
            conv_chunk("q", q_cm, 1, IC_CH, HH, 0, cbos=(0,))
            for ch in range(N_KC):
                conv_chunk("v", v_cm, 2, KC_CH, HK, ch)
            conv_chunk("q", q_cm, 1, IC_CH, HH, 0, cbos=(1,))
            v_tm = pp.tile([128, N_JT, C], BF, tag="vtm", name="vtm")
            for jt in range(N_JT):
                for cb in range(2):
                    tp = psp.tile([128, 512], FP, tag="ps", name="ps")
                    nc.tensor.transpose(
                        tp[:JT, :128],
                        v_cm[cb][:, jt * JT:(jt + 1) * JT],
                        ident[:],
                    )
                    nc.vector.tensor_copy(
                        v_tm[:JT, jt, cb * 128:(cb + 1) * 128], tp[:JT, :128]
                    )

            # ---- phase 2: attention, next q conv chunk prefetched inside ----
            o_cm = [pp.tile([128, N], BF, tag=f"ocm{cb}", name=f"ocm{cb}") for cb in range(2)]
            ti_done = 0
            for ic in range(N_IC):
                for hg in range(2):
                    o_ps = pso.tile([128, IC_F], FP, tag="o", name="o")
                    d_ps = psd.tile([128, IC_F], FP, tag="d", name="d")

                    def s_mm(jt):
                        # one PSUM tile (1 bank) per head: each exp engine's
                        # pipeline (exp -> next S -> exp) is chained per head,
                        # so per-head tiles keep the four chains independent
                        st = [
                            pss.tile([128, 512], FP, tag=f"s{hh}", name=f"s{hh}")
                            for hh in range(4)
                        ]
                        for hh in (0, 2, 1, 3):
                            nc.tensor.matmul(
                                st[hh][:JT, :IC_F],
                                lhsT=(k_cm[hg][hh * 32:(hh + 1) * 32, jt * JT:(jt + 1) * JT]),
                                rhs=(q_cm[hg][hh * 32:(hh + 1) * 32, ic * IC_F:(ic + 1) * IC_F]),
                                start=True,
                                stop=True,
                                tile_position=(32 * hh, 0),
                            )
                        return st

                    st = s_mm(0)
                    for jt in range(N_JT):
                        # exp split: ScalarE native on heads 0..SC_HEADS-1,
                        # DVE Schraudolph bit-exp on the rest
                        et = [None] * 4
                        for hh in (0, 2, 1, 3):
                            e = ep.tile([128, IC_F], BF, tag=f"e{hh}", name=f"e{hh}")
                            if hh < SC_HEADS:
                                nc.scalar.activation(
                                    e[:JT, :], st[hh][:JT, :IC_F], AF.Exp, scale=SCALE
                                )
                            else:
                                nc.vector.tensor_scalar(
                                    e[:JT, :].bitcast(I16),
                                    st[hh][:JT, :IC_F],
                                    SCH_A,
                                    SCH_B,
                                    ALU.mult,
                                    ALU.add,
                                )
                            et[hh] = e
                        if jt + 1 < N_JT:
                            st = s_mm(jt + 1)
                        for hh in (0, 2, 1, 3):
                            nc.tensor.matmul(
                                o_ps[hh * 32:(hh + 1) * 32, :],
                                lhsT=(v_tm[:JT, jt, hg * 128 + hh * 32: hg * 128 + (hh + 1) * 32]),
                                rhs=(et[hh][:JT, :]),
                                start=(jt == 0),
                                stop=(jt == N_JT - 1),
                                tile_position=(0, 32 * hh),
                                skip_group_check=True,
                            )
                            nc.tensor.matmul(
                                d_ps[hh * 32:(hh + 1) * 32, :],
                                lhsT=(ones[:JT, :]),
                                rhs=(et[hh][:JT, :]),
                                start=(jt == 0),
                                stop=(jt == N_JT - 1),
                                tile_position=(0, 32 * hh),
                                skip_group_check=True,
                            )
                    r_t = rp.tile([128, IC_F], FP, tag="r", name="r")
                    nc.vector.reciprocal_approx_fast(r_t[:], d_ps[:])
                    nc.vector.tensor_mul(
                        o_cm[hg][:, ic * IC_F:(ic + 1) * IC_F], o_ps[:], r_t[:]
                    )
                # prefetch next q conv chunk; bias-add on ScalarE so the
                # DVE FIFO (Schraudolph exps) is not blocked behind it
                if ic + 1 < N_IC:
                    conv_chunk("q", q_cm, 1, IC_CH, HH, ic + 1, eng="scalar")

                # ---- token-major proj for tokens that are ready ----
                while ti_done < N_TT and min(ti_done * 128, N - 128) + 128 <= (ic + 1) * IC_F:
                    st = min(ti_done * 128, N - 128)  # overlap the ragged tail
                    po = psp.tile([128, 512], FP, tag="ps", name="ps")
                    for cb in range(2):
                        nc.tensor.matmul(
                            po[:, :C],
                            lhsT=(o_cm[cb][:, st: st + 128]),
                            rhs=(pw2[cb][:]),
                            start=(cb == 0),
                            stop=(cb == 1),
                        )
                    ot = outp.tile([128, C], FP, tag="ot", name="ot")
                    # fused bias add with the PSUM->SBUF copy
                    nc.vector.scalar_tensor_tensor(
                        ot[:], po[:, :C], 1.0, bias_bc[:], ALU.mult, ALU.add
                    )
                    nc.sync.dma_start(out_d[st: st + 128, :], ot[:])
                    ti_done += 1

    nc.compile()
    return nc


def _fold_weights(inp, p):
    scale = inp[f"{p}_bn_g"] / np.sqrt(inp[f"{p}_bn_v"] + EPS)
    shift = inp[f"{p}_bn_b"] - inp[f"{p}_bn_m"] * scale
    w2 = inp[f"{p}_pw_w"] * scale[None, :]          # (o, c)
    w9 = inp[f"{p}_dw_w"].reshape(C, 9)             # (c, tap)
    w9t = np.ascontiguousarray(
        w2.T[None, :, :] * w9.T[:, :, None]          # (tap, c, o)
    ).astype(np.float32)
    const = (
        inp[f"{p}_pw_w"] @ (scale * inp[f"{p}_dw_b"] + shift) + inp[f"{p}_pw_b"]
    ).astype(np.float32)
    return w9t, const.reshape(C, 1)


def _host_inputs(inp):
    common = {}
    for p in ("q", "k", "v"):
        w9t, const = _fold_weights(inp, p)        # (tap, c, o), (C, 1)
        # layout for DoubleRow: [c_in_block(128), cbi(2), tap(9), o(256)]
        w9l = np.ascontiguousarray(
            w9t.reshape(9, 2, 128, C).transpose(2, 1, 0, 3)
        )
        # per-output-channel pow2 scale so fp8e4 (max 240) keeps precision
        mx = np.abs(w9l).max(axis=(0, 1, 2))      # (256,)
        sc = np.exp2(np.floor(np.log2(200.0 / np.maximum(mx, 1e-30))))
        common[f"{p}_w9t8"] = (w9l * sc[None, None, None, :]).astype(NP8)
        # packed [const_cb0, const_cb1, ws_cb0, ws_cb1] as [128, 4] fp32
        cw = np.empty((128, 4), np.float32)
        cw[:, 0:2] = const.reshape(2, 128).T
        cw[:, 2:4] = (1.0 / sc).reshape(2, 128).T
        common[f"{p}_cw"] = cw
    pwt = np.ascontiguousarray(inp["proj_w"].T)      # (c_in, c_out)
    common["proj_w2"] = np.ascontiguousarray(
        pwt.reshape(2, 128, C)
    ).astype(ml_dtypes.bfloat16)
    common["bias_bc"] = np.ascontiguousarray(
        np.broadcast_to(inp["proj_b"].astype(np.float32), (128, C))
    )
    return common


def _x_pad8(xb):
    # (3136, 256) -> fp8 [c_in_block(128), cbi(2), 58, 64] zero-padded image
    xc = xb.reshape(HH, HH, C).transpose(2, 0, 1)    # (256, 56, 56)
    xp = np.zeros((2, 128, PADW, PADW2), np.float32)
    xp[0, :, 1:57, 1:57] = xc[:128]
    xp[1, :, 1:57, 1:57] = xc[128:]
    return np.ascontiguousarray(xp.transpose(1, 0, 2, 3)).astype(NP8)


def _in_maps(inp):
    common = _host_inputs(inp)
    x = np.asarray(inp["x"]).astype(np.float32)
    return [dict(common, x_pad8=_x_pad8(x[b])) for b in range(x.shape[0])]


def kernel(**inputs):
    inp = {k: np.asarray(v) for k, v in inputs.items()}
    B = inp["x"].shape[0]

    if "nc" not in _CACHED:
        _CACHED["nc"] = _build_nc()
    nc = _CACHED["nc"]

    in_maps = _in_maps(inp)
    res = run_bass_kernel_spmd(nc, in_maps, list(range(B)))
    out = np.stack([res.results[b]["out"] for b in range(B)], axis=0)
    return out.astype(np.float32)
